# revision 1
# baseline (speedup 1.0000x reference)
"""RIENet loss kernel (keypoint/KNN MSE + global-align Huber-min loss) on 8 trn2 cores.

Sharding: core ci -> (b = ci // 4, n-chunk j = ci % 4).  Each core holds the full
tgt[b] (M=8192 points) and a 2048-column chunk of src_transformed[b] (N axis).
  loss_1 (min over M per src point): complete locally per core.
  loss_2 (min over N per tgt point): per-core partial min over its chunk;
          host min-reduces the 4 chunks per batch element.

Device kernel per core (v2 — bf16-split matmul, PE off the critical path):
  Q[m, n] = -2 t_m . s_n + ||s_n||^2 computed by one K=21 bf16 matmul:
  t and s are split 3-way into bf16 (hi/mid/lo, ~27 mantissa bits total) and
  the 6 dominant cross products are taken (error ~1e-6 absolute); ||s||^2 is
  split 3-way against ones-rows.  ||t_m||^2 stays fp32 and is folded in
  per-partition by scalar_tensor_tensor during the column-min accumulation:
    acc = min(Q + nt[m], acc)           (min over m-tiles, DVE, one pass)
    rowbuf[:, mi] = reduce_min(Q)       (min over n-chunk, DVE, one pass)
  rowbuf gets nt added at the end; acc is partition-min-reduced via PE
  transposes.  Tiny keypoint/KNN MSE losses run on-device on every core.
"""

import os
import numpy as np


def _ensure_path():
    try:
        import concourse  # noqa: F401
    except ImportError:
        import sys
        for p in ("/opt/trn_rl_repo", "/root/.axon_site/_ro/trn_rl_repo"):
            if os.path.isdir(p) and p not in sys.path:
                sys.path.insert(0, p)


_ensure_path()

import concourse.bass as bass  # noqa: E402
import concourse.bacc as bacc  # noqa: E402
import concourse.tile as tile  # noqa: E402
import concourse.mybir as mybir  # noqa: E402
from concourse.bass_utils import run_bass_kernel_spmd  # noqa: E402

F32 = mybir.dt.float32
BF16 = mybir.dt.bfloat16
AL = mybir.AluOpType
AF = mybir.ActivationFunctionType

MARGIN = 0.1
B, KP, KNN, N, M = 2, 256, 32, 8192, 8192
NCORES = 8
NSHARDS = NCORES // B          # 4 n-chunks per batch element
CHUNK = N // NSHARDS           # 2048
NJ = CHUNK // 512              # 4 psum banks per m-tile
MI = M // 128                  # 64 m-tiles
GT = M // 128                  # 64 groups in the [p, d, g] tgt layout
GS = CHUNK // 128              # 16 groups in the [p, d, g] src layout
K21 = 21
BIG = 3.0e38

_CACHE = {}


def _build():
    nc = bacc.Bacc("TRN2", target_bir_lowering=False, debug=False,
                   num_devices=NCORES)

    src = nc.dram_tensor("src", [3, CHUNK], F32, kind="ExternalInput")
    tgt = nc.dram_tensor("tgt", [3, M], F32, kind="ExternalInput")
    ident = nc.dram_tensor("ident", [128, 128], F32, kind="ExternalInput")
    kp_lhsT = nc.dram_tensor("kp_lhsT", [4, 2 * 3], F32, kind="ExternalInput")
    kp_rhs = nc.dram_tensor("kp_rhs", [4, 2 * KP], F32, kind="ExternalInput")
    tgt_kp = nc.dram_tensor("tgt_kp", [3, 2 * KP], F32, kind="ExternalInput")
    knn_src = nc.dram_tensor("knn_src", [128, 2 * 192], F32, kind="ExternalInput")
    knn_tgt = nc.dram_tensor("knn_tgt", [128, 2 * 192], F32, kind="ExternalInput")

    colmin_o = nc.dram_tensor("colmin", [128, CHUNK // 128], F32, kind="ExternalOutput")
    rowmin_o = nc.dram_tensor("rowmin", [128, MI], F32, kind="ExternalOutput")
    misc_o = nc.dram_tensor("misc", [128, 4], F32, kind="ExternalOutput")

    with tile.TileContext(nc) as tc:
        with (
            tc.tile_pool(name="const", bufs=1) as const,
            tc.tile_pool(name="sc", bufs=3) as sc,
        ):
            tA = const.tile([K21, M], BF16)       # lhsT rows
            sA = const.tile([K21, CHUNK], BF16)   # rhs rows
            acc = const.tile([128, CHUNK], F32)
            rowbuf = const.tile([128, MI], F32)
            nt_all = const.tile([128, GT], F32)   # ||t||^2, [p, mi]
            id_sb = const.tile([128, 128], F32)
            colmin_sb = const.tile([128, CHUNK // 128], F32)
            misc_sb = const.tile([128, 4], F32)

            nc.sync.dma_start(out=id_sb[:], in_=ident[:])
            nc.gpsimd.memset(acc[:], BIG)
            nc.gpsimd.memset(misc_sb[:], 0.0)

            # ---- load t, s in [p, d, g] layouts (partition-minor DMA) ----
            tw = const.tile([128, 3, GT], F32)
            sw = const.tile([128, 3, GS], F32)
            nc.sync.dma_start(out=tw[:], in_=tgt.rearrange("d (g p) -> p d g", p=128))
            nc.sync.dma_start(out=sw[:], in_=src.rearrange("d (g p) -> p d g", p=128))

            # ---- norms (fp32) ----
            tsq = const.tile([128, 3, GT], F32)
            nc.vector.tensor_mul(tsq[:], tw[:], tw[:])
            nc.vector.tensor_add(nt_all[:], tsq[:, 0, :], tsq[:, 1, :])
            nc.vector.tensor_add(nt_all[:], nt_all[:], tsq[:, 2, :])
            ssq = const.tile([128, 3, GS], F32)
            ns_w = const.tile([128, GS], F32)
            nc.vector.tensor_mul(ssq[:], sw[:], sw[:])
            nc.vector.tensor_add(ns_w[:], ssq[:, 0, :], ssq[:, 1, :])
            nc.vector.tensor_add(ns_w[:], ns_w[:], ssq[:, 2, :])

            # ---- 3-way bf16 splits (kept as exactly-rounded fp32 tiles) ----
            nc.scalar.mul(out=tw[:], in_=tw[:], mul=-2.0)  # fold -2 into t side

            def split3(name, w, shape):
                outs = []
                cur = w
                for lvl in range(3):
                    b16 = sc.tile(shape, BF16, tag=f"{name}_b{lvl}")
                    nc.scalar.copy(out=b16[:], in_=cur[:])
                    f32t = const.tile(shape, F32, tag=f"{name}_f{lvl}")
                    nc.vector.tensor_copy(out=f32t[:], in_=b16[:])
                    outs.append(f32t)
                    if lvl < 2:
                        nxt = const.tile(shape, F32, tag=f"{name}_r{lvl}")
                        nc.vector.tensor_sub(nxt[:], cur[:], f32t[:])
                        cur = nxt
                return outs

            th, tm, tl = split3("t", tw, [128, 3, GT])
            sh, sm, sl = split3("s", sw, [128, 3, GS])
            nsp = split3("n", ns_w.rearrange("p (o g) -> p o g", o=1),
                         [128, 1, GS])

            # pairing layout: lhsT rows [th,th,tm,tm,th,tl]*3d + ones*3
            #                 rhs  rows [sh,sm,sh,sm,sl,sh]*3d + ns_splits
            t_dest = {0: [0, 3, 12], 1: [6, 9], 2: [15]}    # th, tm, tl
            s_dest = {0: [0, 6, 15], 1: [3, 9], 2: [12]}    # sh, sm, sl

            with tc.tile_pool(name="psum_pre", bufs=4, space="PSUM") as pp:
                def place(w_f32, groups, dst_tile, rows, width):
                    # transpose [128, g] -> [g, 128] via PE, cast to bf16,
                    # then DMA into row(s) of the operand tile
                    pt = pp.tile([groups, 128], F32, tag=f"tp{groups}")
                    nc.tensor.transpose(pt[:], w_f32, id_sb[:])
                    tr = sc.tile([groups, 128], BF16, tag=f"tr{groups}")
                    nc.scalar.copy(out=tr[:], in_=pt[:])
                    for r in rows:
                        nc.sync.dma_start(
                            out=dst_tile[r:r + 1, :].rearrange(
                                "o (g p) -> o g p", p=128),
                            in_=tr[:])

                for lvl, w in enumerate([th, tm, tl]):
                    for d in range(3):
                        place(w[:, d, :], GT, tA,
                              [base + d for base in t_dest[lvl]], M)
                for lvl, w in enumerate([sh, sm, sl]):
                    for d in range(3):
                        place(w[:, d, :], GS, sA,
                              [base + d for base in s_dest[lvl]], CHUNK)
                for lvl in range(3):
                    place(nsp[lvl][:, 0, :], GS, sA, [18 + lvl], CHUNK)

                # ones rows 18-20 of lhsT (staged at partition 0, DMA'd up)
                ones3 = const.tile([3, M], BF16)
                nc.vector.memset(ones3[:], 1.0)
                nc.sync.dma_start(out=tA[18:21, :], in_=ones3[:])

            # ---- main loop: Q = -2 t.s + ||s||^2 per 128-row m-tile ----
            with tc.tile_pool(name="psum_main", bufs=2, space="PSUM") as pm:
                for mi in range(MI):
                    pt = pm.tile([128, CHUNK], F32, tag="pt")
                    for nj in range(NJ):
                        nc.tensor.matmul(
                            pt[:, nj * 512:(nj + 1) * 512],
                            lhsT=tA[:, mi * 128:(mi + 1) * 128],
                            rhs=sA[:, nj * 512:(nj + 1) * 512],
                            start=True, stop=True,
                        )
                    # colmin: acc = min(Q + nt[m], acc)
                    nc.vector.scalar_tensor_tensor(
                        out=acc[:], in0=pt[:], scalar=nt_all[:, mi:mi + 1],
                        in1=acc[:], op0=AL.add, op1=AL.min)
                    # rowmin over the n-chunk (nt added after the loop)
                    nc.vector.tensor_reduce(
                        out=rowbuf[:, mi:mi + 1], in_=pt[:],
                        axis=mybir.AxisListType.X, op=AL.min)

            nc.vector.tensor_add(rowbuf[:], rowbuf[:], nt_all[:])

            with tc.tile_pool(name="psum_fin", bufs=2, space="PSUM") as pf:
                # partition-axis min of acc via PE transposes
                for blk in range(CHUNK // 128):
                    tp = pf.tile([128, 128], F32, tag="tp")
                    nc.tensor.transpose(tp[:], acc[:, blk * 128:(blk + 1) * 128],
                                        id_sb[:])
                    nc.vector.tensor_reduce(
                        out=colmin_sb[:, blk:blk + 1], in_=tp[:],
                        axis=mybir.AxisListType.X, op=AL.min)

                # tiny keypoint / knn losses (both batch elements)
                kp_l = const.tile([4, 2 * 3], F32)
                kp_r = const.tile([4, 2 * KP], F32)
                kp_t = const.tile([3, 2 * KP], F32)
                ks = const.tile([128, 2 * 192], F32)
                kt = const.tile([128, 2 * 192], F32)
                nc.sync.dma_start(out=kp_l[:], in_=kp_lhsT[:])
                nc.sync.dma_start(out=kp_r[:], in_=kp_rhs[:])
                nc.sync.dma_start(out=kp_t[:], in_=tgt_kp[:])
                nc.sync.dma_start(out=ks[:], in_=knn_src[:])
                nc.sync.dma_start(out=kt[:], in_=knn_tgt[:])
                for b in range(B):
                    pt2 = pf.tile([3, KP], F32, tag="kp")
                    nc.tensor.matmul(
                        pt2[:], lhsT=kp_l[:, b * 3:(b + 1) * 3],
                        rhs=kp_r[:, b * KP:(b + 1) * KP],
                        start=True, stop=True)
                    diff = sc.tile([3, KP], F32, tag="kdiff")
                    nc.vector.tensor_sub(diff[:], pt2[:],
                                         kp_t[:, b * KP:(b + 1) * KP])
                    nc.vector.tensor_mul(diff[:], diff[:], diff[:])
                    nc.vector.tensor_reduce(
                        out=misc_sb[0:3, b:b + 1], in_=diff[:],
                        axis=mybir.AxisListType.X, op=AL.add)
                    diff2 = sc.tile([128, 192], F32, tag="ndiff")
                    nc.vector.tensor_sub(diff2[:], ks[:, b * 192:(b + 1) * 192],
                                         kt[:, b * 192:(b + 1) * 192])
                    nc.vector.tensor_mul(diff2[:], diff2[:], diff2[:])
                    nc.vector.tensor_reduce(
                        out=misc_sb[:, 2 + b:3 + b], in_=diff2[:],
                        axis=mybir.AxisListType.X, op=AL.add)

            nc.sync.dma_start(out=colmin_o[:], in_=colmin_sb[:])
            nc.sync.dma_start(out=rowmin_o[:], in_=rowbuf[:])
            nc.sync.dma_start(out=misc_o[:], in_=misc_sb[:])

    nc.compile()
    return nc


def _get_nc():
    if "nc" not in _CACHE:
        _CACHE["nc"] = _build()
    return _CACHE["nc"]


def _prepare_in_maps(src_keypoints, tgt_keypoints, rotation_ab, translation_ab,
                     src_keypoints_knn, tgt_keypoints_knn, src_transformed, tgt):
    f = np.float32
    st = np.ascontiguousarray(np.asarray(src_transformed, dtype=f))
    tg = np.ascontiguousarray(np.asarray(tgt, dtype=f))
    skp = np.asarray(src_keypoints, dtype=f)
    tkp = np.asarray(tgt_keypoints, dtype=f)
    rot = np.asarray(rotation_ab, dtype=f)
    tra = np.asarray(translation_ab, dtype=f)
    sknn = np.asarray(src_keypoints_knn, dtype=f)
    tknn = np.asarray(tgt_keypoints_knn, dtype=f)

    ident = np.eye(128, dtype=f)
    kp_lhsT = np.zeros((4, 2 * 3), dtype=f)
    kp_rhs = np.zeros((4, 2 * KP), dtype=f)
    tgt_kp = np.zeros((3, 2 * KP), dtype=f)
    knn_src = np.zeros((128, 2 * 192), dtype=f)
    knn_tgt = np.zeros((128, 2 * 192), dtype=f)
    for b in range(B):
        kp_lhsT[0:3, b * 3:(b + 1) * 3] = rot[b].T
        kp_lhsT[3, b * 3:(b + 1) * 3] = tra[b]
        kp_rhs[0:3, b * KP:(b + 1) * KP] = skp[b]
        kp_rhs[3, b * KP:(b + 1) * KP] = 1.0
        tgt_kp[:, b * KP:(b + 1) * KP] = tkp[b]
        knn_src[:, b * 192:(b + 1) * 192] = sknn[b].reshape(128, 192)
        knn_tgt[:, b * 192:(b + 1) * 192] = tknn[b].reshape(128, 192)

    shared = {
        "ident": ident, "kp_lhsT": kp_lhsT, "kp_rhs": kp_rhs,
        "tgt_kp": tgt_kp, "knn_src": knn_src, "knn_tgt": knn_tgt,
    }
    in_maps = []
    for ci in range(NCORES):
        b, j = divmod(ci, NSHARDS)
        m = dict(shared)
        m["src"] = np.ascontiguousarray(st[b, :, j * CHUNK:(j + 1) * CHUNK])
        m["tgt"] = tg[b]
        in_maps.append(m)
    return in_maps


def _huber(x, c):
    return np.where(x < c, 0.5 * x * x, c * x - 0.5 * c * c)


def _postprocess(results):
    c = np.float64(MARGIN)
    loss1 = np.float64(0.0)
    loss2 = np.float64(0.0)
    for b in range(B):
        rowmins = []
        for j in range(NSHARDS):
            r = results[b * NSHARDS + j]
            colmin = np.asarray(r["colmin"], dtype=np.float64).T.ravel()
            loss1 += _huber(colmin, c).sum()
            rowmins.append(np.asarray(r["rowmin"], dtype=np.float64).T.ravel())
        rm = np.minimum.reduce(rowmins)
        loss2 += _huber(rm, c).sum()
    gal = loss1 + loss2

    misc = np.asarray(results[0]["misc"], dtype=np.float64)
    kp_loss = (misc[0:3, 0].sum() + misc[0:3, 1].sum()) / B
    knn_loss = (misc[:, 2].sum() + misc[:, 3].sum()) / (B * KNN)
    ncl = knn_loss + kp_loss
    return np.float32(ncl), np.float32(gal)


def run_device(in_maps, **kw):
    nc = _get_nc()
    return run_bass_kernel_spmd(nc, in_maps, list(range(NCORES)), **kw)


def kernel(src_keypoints, tgt_keypoints, rotation_ab, translation_ab,
           src_keypoints_knn, tgt_keypoints_knn, k, src_transformed, tgt,
           **_unused):
    in_maps = _prepare_in_maps(src_keypoints, tgt_keypoints, rotation_ab,
                               translation_ab, src_keypoints_knn,
                               tgt_keypoints_knn, src_transformed, tgt)
    res = run_device(in_maps)
    return _postprocess(res.results)



# revision 8
# speedup vs baseline: 1.5315x; 1.5315x over previous
"""RIENet loss kernel (keypoint/KNN MSE + global-align Huber-min loss) on 8 trn2 cores.

Sharding: core ci -> (b = ci // 4, n-chunk j = ci % 4).  Each core holds the full
tgt[b] (M=8192 points) and a 2048-column chunk of src_transformed[b] (N axis).
  loss_1 (min over M per src point): complete locally per core.
  loss_2 (min over N per tgt point): per-core partial min over its chunk;
          host min-reduces the 4 chunks per batch element.

v3 design (three-engine pipeline, host-side operand prep):
  Host builds bf16-split operand matrices tA [24, M] / sA [24, CHUNK] so that a
  single K=24 bf16 matmul produces the FULL squared-distance matrix
    P[m, n] = ||t_m||^2 + ||s_n||^2 - 2 t_m.s_n
  in fp32 PSUM (3-way bf16 splits of t, s and both norms; the 6 dominant
  cross products; abs err ~1e-5).  No device-side splits/transposes.

  Per 128-row m-tile (64 tiles):
    PE:   4 x 512-col bf16 matmuls -> PSUM [128, 2048]
    ACT:  copy PSUM -> SBUF bf16 (the only engine that is otherwise idle)
    DVE:  tensor_tensor min          acc_bf = min(acc_bf, q_bf)   (colmin, 2x)
          tensor_mask_reduce min     rowbuf[:, mi] = min_n q_bf   (rowmin)
  Tail: partition-min of acc_bf via 16 PE transposes + DVE reduces.
  Tiny keypoint/KNN MSE losses are computed on host in float64.
"""

import os
import numpy as np


def _ensure_path():
    try:
        import concourse  # noqa: F401
    except ImportError:
        import sys
        for p in ("/opt/trn_rl_repo", "/root/.axon_site/_ro/trn_rl_repo"):
            if os.path.isdir(p) and p not in sys.path:
                sys.path.insert(0, p)


_ensure_path()

import concourse.bass as bass  # noqa: E402
import concourse.bacc as bacc  # noqa: E402
import concourse.tile as tile  # noqa: E402
import concourse.mybir as mybir  # noqa: E402
from concourse.bass_utils import run_bass_kernel_spmd  # noqa: E402

F32 = mybir.dt.float32
BF16 = mybir.dt.bfloat16
AL = mybir.AluOpType

MARGIN = 0.1
B, KP, KNN, N, M = 2, 256, 32, 8192, 8192
NCORES = 8
NSHARDS = NCORES // B          # 4 n-chunks per batch element
CHUNK = N // NSHARDS           # 2048
NJ = CHUNK // 512              # 4 psum banks per m-tile
MI = M // 128                  # 64 m-tiles
K24 = 24
BIG = 3.0e38

_CACHE = {}
_BF16_NP = mybir.dt.np(BF16)


def _build():
    nc = bacc.Bacc("TRN2", target_bir_lowering=False, debug=False,
                   num_devices=NCORES)

    tA_d = nc.dram_tensor("tA", [K24, M], BF16, kind="ExternalInput")
    sA_d = nc.dram_tensor("sA", [K24, CHUNK], BF16, kind="ExternalInput")
    ident = nc.dram_tensor("ident", [128, 128], F32, kind="ExternalInput")

    colmin_o = nc.dram_tensor("colmin", [128, CHUNK // 128], F32,
                              kind="ExternalOutput")
    rowmin_o = nc.dram_tensor("rowmin", [128, MI], F32, kind="ExternalOutput")

    with tile.TileContext(nc) as tc:
        with (
            tc.tile_pool(name="const", bufs=1) as const,
            tc.tile_pool(name="qp", bufs=2) as qp,
        ):
            tA = const.tile([K24, M], BF16)
            sA = const.tile([K24, CHUNK], BF16)
            id_sb = const.tile([128, 128], F32)
            accf32 = const.tile([128, CHUNK], F32)
            accbf = const.tile([128, CHUNK], BF16)
            scr = const.tile([128, CHUNK], BF16)
            rowbuf = const.tile([128, MI], F32)
            colmin_sb = const.tile([128, CHUNK // 128], F32)
            me_ap = const.tile([128, 1], F32)

            nc.sync.dma_start(out=tA[:], in_=tA_d[:])
            nc.sync.dma_start(out=sA[:], in_=sA_d[:])
            nc.sync.dma_start(out=id_sb[:], in_=ident[:])
            nc.gpsimd.memset(accbf[:], BIG)
            nc.gpsimd.memset(me_ap[:], float(CHUNK))

            # ---- main loop: P = nt + ns - 2 t.s per 128-row m-tile ----
            with tc.tile_pool(name="psum_main", bufs=2, space="PSUM") as pm:
                for mi in range(MI):
                    pt = pm.tile([128, CHUNK], F32, tag="pt")
                    for nj in range(NJ):
                        nc.tensor.matmul(
                            pt[:, nj * 512:(nj + 1) * 512],
                            lhsT=tA[:, mi * 128:(mi + 1) * 128],
                            rhs=sA[:, nj * 512:(nj + 1) * 512],
                            start=True, stop=True,
                        )
                    qbf = qp.tile([128, CHUNK], BF16, tag="qbf")
                    nc.scalar.copy(out=qbf[:], in_=pt[:])
                    # rowmin over the n-chunk
                    nc.vector.tensor_reduce(
                        out=rowbuf[:, mi:mi + 1], in_=qbf[:],
                        axis=mybir.AxisListType.X, op=AL.min)
                    # colmin partial: acc = min(acc, q)
                    nc.vector.tensor_tensor(
                        out=accbf[:], in0=qbf[:], in1=accbf[:], op=AL.min)

            with tc.tile_pool(name="psum_fin", bufs=2, space="PSUM") as pf:
                # partition-axis min of accbf via PE transposes (fp32 path)
                nc.scalar.copy(out=accf32[:], in_=accbf[:])
                for blk in range(CHUNK // 128):
                    tp = pf.tile([128, 128], F32, tag="tp")
                    nc.tensor.transpose(tp[:], accf32[:, blk * 128:(blk + 1) * 128],
                                        id_sb[:])
                    nc.vector.tensor_reduce(
                        out=colmin_sb[:, blk:blk + 1], in_=tp[:],
                        axis=mybir.AxisListType.X, op=AL.min)

            nc.sync.dma_start(out=colmin_o[:], in_=colmin_sb[:])
            nc.sync.dma_start(out=rowmin_o[:], in_=rowbuf[:])

    nc.compile()
    return nc


def _get_nc():
    if "nc" not in _CACHE:
        _CACHE["nc"] = _build()
    return _CACHE["nc"]


def _split3(x):
    """3-way bf16 split of a float array (computed in float64/float32)."""
    x = np.asarray(x, dtype=np.float64)
    h = x.astype(_BF16_NP)
    r1 = x - h.astype(np.float64)
    m = r1.astype(_BF16_NP)
    r2 = r1 - m.astype(np.float64)
    l = r2.astype(_BF16_NP)
    return h, m, l


def _prepare_in_maps(src_transformed, tgt):
    f = np.float32
    st = np.asarray(src_transformed, dtype=f)
    tg = np.asarray(tgt, dtype=f)

    ident = np.eye(128, dtype=f)

    in_maps = []
    for ci in range(NCORES):
        b, j = divmod(ci, NSHARDS)
        t = tg[b]                                    # (3, M)
        s = st[b, :, j * CHUNK:(j + 1) * CHUNK]      # (3, CHUNK)

        th, tm, tl = _split3(t)
        sh, sm, sl = _split3(s)
        nt = np.sum(t.astype(np.float64) ** 2, axis=0)
        ns = np.sum(s.astype(np.float64) ** 2, axis=0)
        nth, ntm, ntl = _split3(nt)
        nsh, nsm, nsl = _split3(ns)

        def neg2(a):
            return (a.astype(f) * -2.0).astype(_BF16_NP)

        tA = np.zeros((K24, M), dtype=_BF16_NP)
        sA = np.zeros((K24, CHUNK), dtype=_BF16_NP)
        # cross products: (th,sh) (th,sm) (tm,sh) (tm,sm) (th,sl) (tl,sh)
        pairs = [(th, sh), (th, sm), (tm, sh), (tm, sm), (th, sl), (tl, sh)]
        r = 0
        for tp_, sp_ in pairs:
            for d in range(3):
                tA[r] = neg2(tp_[d])
                sA[r] = sp_[d]
                r += 1
        # ||s||^2 rows: ones x ns splits
        for part in (nsh, nsm, nsl):
            tA[r] = np.ones(M, dtype=_BF16_NP)
            sA[r] = part
            r += 1
        # ||t||^2 rows: nt splits x ones
        for part in (nth, ntm, ntl):
            tA[r] = part
            sA[r] = np.ones(CHUNK, dtype=_BF16_NP)
            r += 1
        assert r == K24

        in_maps.append({
            "tA": np.ascontiguousarray(tA),
            "sA": np.ascontiguousarray(sA),
            "ident": ident,
        })
    return in_maps


def _huber(x, c):
    return np.where(x < c, 0.5 * x * x, c * x - 0.5 * c * c)


def _postprocess(results):
    c = np.float64(MARGIN)
    loss1 = np.float64(0.0)
    loss2 = np.float64(0.0)
    for b in range(B):
        rowmins = []
        for j in range(NSHARDS):
            r = results[b * NSHARDS + j]
            colmin = np.asarray(r["colmin"], dtype=np.float64).T.ravel()
            loss1 += _huber(colmin, c).sum()
            rowmins.append(np.asarray(r["rowmin"], dtype=np.float64).T.ravel())
        rm = np.minimum.reduce(rowmins)
        loss2 += _huber(rm, c).sum()
    return loss1 + loss2


def run_device(in_maps, **kw):
    nc = _get_nc()
    return run_bass_kernel_spmd(nc, in_maps, list(range(NCORES)), **kw)


def _ncl_host(src_keypoints, tgt_keypoints, rotation_ab, translation_ab,
              src_keypoints_knn, tgt_keypoints_knn):
    f64 = np.float64
    skp = np.asarray(src_keypoints, dtype=f64)
    tkp = np.asarray(tgt_keypoints, dtype=f64)
    rot = np.asarray(rotation_ab, dtype=f64)
    tra = np.asarray(translation_ab, dtype=f64)
    sknn = np.asarray(src_keypoints_knn, dtype=f64)
    tknn = np.asarray(tgt_keypoints_knn, dtype=f64)

    transformed = np.einsum('bij,bjk->bik', rot, skp) + tra[:, :, None]
    kp_sq = (transformed - tkp) ** 2
    keypoints_loss = np.mean(np.sum(kp_sq, axis=(1, 2)))
    knn_sq = (sknn - tknn) ** 2
    knn_loss = np.mean(np.sum(knn_sq, axis=(1, 2)))
    return knn_loss + keypoints_loss


def kernel(src_keypoints, tgt_keypoints, rotation_ab, translation_ab,
           src_keypoints_knn, tgt_keypoints_knn, k, src_transformed, tgt,
           **_unused):
    in_maps = _prepare_in_maps(src_transformed, tgt)
    res = run_device(in_maps)
    gal = _postprocess(res.results)
    ncl = _ncl_host(src_keypoints, tgt_keypoints, rotation_ab, translation_ab,
                    src_keypoints_knn, tgt_keypoints_knn)
    return np.float32(ncl), np.float32(gal)


# revision 10
# speedup vs baseline: 1.9967x; 1.3038x over previous
"""RIENet loss kernel (keypoint/KNN MSE + global-align Huber-min loss) on 8 trn2 cores.

Sharding: core ci -> (b = ci // 4, n-chunk j = ci % 4).  Each core holds the full
tgt[b] (M=8192 points) and a 2048-column chunk of src_transformed[b] (N axis).
  loss_1 (min over M per src point): complete locally per core.
  loss_2 (min over N per tgt point): per-core partial min over its chunk;
          host min-reduces the 4 chunks per batch element.

v3 design (three-engine pipeline, host-side operand prep):
  Host builds bf16-split operand matrices tA [24, M] / sA [24, CHUNK] so that a
  single K=24 bf16 matmul produces the FULL squared-distance matrix
    P[m, n] = ||t_m||^2 + ||s_n||^2 - 2 t_m.s_n
  in fp32 PSUM (3-way bf16 splits of t, s and both norms; the 6 dominant
  cross products; abs err ~1e-5).  No device-side splits/transposes.

  Per 128-row m-tile (64 tiles):
    PE:   4 x 512-col bf16 matmuls -> PSUM [128, 2048]
    ACT:  copy PSUM -> SBUF bf16 (the only engine that is otherwise idle)
    DVE:  tensor_tensor min          acc_bf = min(acc_bf, q_bf)   (colmin, 2x)
          tensor_mask_reduce min     rowbuf[:, mi] = min_n q_bf   (rowmin)
  Tail: partition-min of acc_bf via 16 PE transposes + DVE reduces.
  Tiny keypoint/KNN MSE losses are computed on host in float64.
"""

import os
import numpy as np


def _ensure_path():
    try:
        import concourse  # noqa: F401
    except ImportError:
        import sys
        for p in ("/opt/trn_rl_repo", "/root/.axon_site/_ro/trn_rl_repo"):
            if os.path.isdir(p) and p not in sys.path:
                sys.path.insert(0, p)


_ensure_path()

import concourse.bass as bass  # noqa: E402
import concourse.bacc as bacc  # noqa: E402
import concourse.tile as tile  # noqa: E402
import concourse.mybir as mybir  # noqa: E402
from concourse.bass_utils import run_bass_kernel_spmd  # noqa: E402

F32 = mybir.dt.float32
BF16 = mybir.dt.bfloat16
AL = mybir.AluOpType

MARGIN = 0.1
B, KP, KNN, N, M = 2, 256, 32, 8192, 8192
NCORES = 8
NSHARDS = NCORES // B          # 4 n-chunks per batch element
CHUNK = N // NSHARDS           # 2048
NJ = CHUNK // 512              # 4 psum banks per m-tile
MI = M // 128                  # 64 m-tiles
K24 = 24
BIG = 3.0e38

_CACHE = {}
_BF16_NP = mybir.dt.np(BF16)


def _build():
    nc = bacc.Bacc("TRN2", target_bir_lowering=False, debug=False,
                   num_devices=NCORES)

    tA_d = nc.dram_tensor("tA", [K24, M], BF16, kind="ExternalInput")
    sA_d = nc.dram_tensor("sA", [K24, CHUNK], BF16, kind="ExternalInput")
    ident = nc.dram_tensor("ident", [128, 128], F32, kind="ExternalInput")

    colmin_o = nc.dram_tensor("colmin", [128, CHUNK // 128], F32,
                              kind="ExternalOutput")
    rowmin_o = nc.dram_tensor("rowmin", [128, MI], F32, kind="ExternalOutput")

    with tile.TileContext(nc) as tc:
        with (
            tc.tile_pool(name="const", bufs=1) as const,
            tc.tile_pool(name="qp", bufs=2) as qp,
        ):
            tA = const.tile([K24, M], BF16)
            sA = const.tile([K24, CHUNK], BF16)
            id_sb = const.tile([128, 128], F32)
            accf32 = const.tile([128, CHUNK], F32)
            accbf = const.tile([128, CHUNK], BF16)
            rowq = const.tile([128, MI, 1024], BF16)
            rowbuf = const.tile([128, MI], F32)
            colmin_sb = const.tile([128, CHUNK // 128], F32)

            nc.sync.dma_start(out=tA[:], in_=tA_d[:])
            nc.sync.dma_start(out=sA[:], in_=sA_d[:])
            nc.sync.dma_start(out=id_sb[:], in_=ident[:])
            nc.gpsimd.memset(accbf[:], BIG)

            # ---- main loop: P = nt + ns - 2 t.s per 128-row m-tile ----
            with tc.tile_pool(name="psum_main", bufs=2, space="PSUM") as pm:
                for mi in range(MI):
                    pt = pm.tile([128, CHUNK], F32, tag="pt")
                    for nj in range(NJ):
                        nc.tensor.matmul(
                            pt[:, nj * 512:(nj + 1) * 512],
                            lhsT=tA[:, mi * 128:(mi + 1) * 128],
                            rhs=sA[:, nj * 512:(nj + 1) * 512],
                            start=True, stop=True,
                        )
                    qbf = qp.tile([128, CHUNK], BF16, tag="qbf")
                    nc.scalar.copy(out=qbf[:], in_=pt[:])
                    # rowmin fold 2048 -> 1024 into this tile's rowq slot
                    nc.vector.tensor_tensor(
                        out=rowq[:, mi, :], in0=qbf[:, 0:1024],
                        in1=qbf[:, 1024:2048], op=AL.min)
                    # colmin partial: acc = min(acc, q)
                    nc.vector.tensor_tensor(
                        out=accbf[:], in0=qbf[:], in1=accbf[:], op=AL.min)

                # batched rowmin: halve rowq in place, then one 3D reduce
                w = 1024
                while w > 8:
                    h = w // 2
                    nc.vector.tensor_tensor(
                        out=rowq[:, :, 0:h], in0=rowq[:, :, 0:h],
                        in1=rowq[:, :, h:w], op=AL.min)
                    w = h
                nc.vector.tensor_reduce(
                    out=rowbuf[:], in_=rowq[:, :, 0:8],
                    axis=mybir.AxisListType.X, op=AL.min)

            with tc.tile_pool(name="psum_fin", bufs=2, space="PSUM") as pf:
                # partition-axis min of accbf via PE transposes (fp32 path)
                nc.scalar.copy(out=accf32[:], in_=accbf[:])
                for blk in range(CHUNK // 128):
                    tp = pf.tile([128, 128], F32, tag="tp")
                    nc.tensor.transpose(tp[:], accf32[:, blk * 128:(blk + 1) * 128],
                                        id_sb[:])
                    nc.vector.tensor_reduce(
                        out=colmin_sb[:, blk:blk + 1], in_=tp[:],
                        axis=mybir.AxisListType.X, op=AL.min)

            nc.sync.dma_start(out=colmin_o[:], in_=colmin_sb[:])
            nc.sync.dma_start(out=rowmin_o[:], in_=rowbuf[:])

    nc.compile()
    return nc


def _get_nc():
    if "nc" not in _CACHE:
        _CACHE["nc"] = _build()
    return _CACHE["nc"]


def _split3(x):
    """3-way bf16 split of a float array (computed in float64/float32)."""
    x = np.asarray(x, dtype=np.float64)
    h = x.astype(_BF16_NP)
    r1 = x - h.astype(np.float64)
    m = r1.astype(_BF16_NP)
    r2 = r1 - m.astype(np.float64)
    l = r2.astype(_BF16_NP)
    return h, m, l


def _prepare_in_maps(src_transformed, tgt):
    f = np.float32
    st = np.asarray(src_transformed, dtype=f)
    tg = np.asarray(tgt, dtype=f)

    ident = np.eye(128, dtype=f)

    in_maps = []
    for ci in range(NCORES):
        b, j = divmod(ci, NSHARDS)
        t = tg[b]                                    # (3, M)
        s = st[b, :, j * CHUNK:(j + 1) * CHUNK]      # (3, CHUNK)

        th, tm, tl = _split3(t)
        sh, sm, sl = _split3(s)
        nt = np.sum(t.astype(np.float64) ** 2, axis=0)
        ns = np.sum(s.astype(np.float64) ** 2, axis=0)
        nth, ntm, ntl = _split3(nt)
        nsh, nsm, nsl = _split3(ns)

        def neg2(a):
            return (a.astype(f) * -2.0).astype(_BF16_NP)

        tA = np.zeros((K24, M), dtype=_BF16_NP)
        sA = np.zeros((K24, CHUNK), dtype=_BF16_NP)
        # cross products: (th,sh) (th,sm) (tm,sh) (tm,sm) (th,sl) (tl,sh)
        pairs = [(th, sh), (th, sm), (tm, sh), (tm, sm), (th, sl), (tl, sh)]
        r = 0
        for tp_, sp_ in pairs:
            for d in range(3):
                tA[r] = neg2(tp_[d])
                sA[r] = sp_[d]
                r += 1
        # ||s||^2 rows: ones x ns splits
        for part in (nsh, nsm, nsl):
            tA[r] = np.ones(M, dtype=_BF16_NP)
            sA[r] = part
            r += 1
        # ||t||^2 rows: nt splits x ones
        for part in (nth, ntm, ntl):
            tA[r] = part
            sA[r] = np.ones(CHUNK, dtype=_BF16_NP)
            r += 1
        assert r == K24

        in_maps.append({
            "tA": np.ascontiguousarray(tA),
            "sA": np.ascontiguousarray(sA),
            "ident": ident,
        })
    return in_maps


def _huber(x, c):
    return np.where(x < c, 0.5 * x * x, c * x - 0.5 * c * c)


def _postprocess(results):
    c = np.float64(MARGIN)
    loss1 = np.float64(0.0)
    loss2 = np.float64(0.0)
    for b in range(B):
        rowmins = []
        for j in range(NSHARDS):
            r = results[b * NSHARDS + j]
            colmin = np.asarray(r["colmin"], dtype=np.float64).T.ravel()
            loss1 += _huber(colmin, c).sum()
            rowmins.append(np.asarray(r["rowmin"], dtype=np.float64).T.ravel())
        rm = np.minimum.reduce(rowmins)
        loss2 += _huber(rm, c).sum()
    return loss1 + loss2


def run_device(in_maps, **kw):
    nc = _get_nc()
    return run_bass_kernel_spmd(nc, in_maps, list(range(NCORES)), **kw)


def _ncl_host(src_keypoints, tgt_keypoints, rotation_ab, translation_ab,
              src_keypoints_knn, tgt_keypoints_knn):
    f64 = np.float64
    skp = np.asarray(src_keypoints, dtype=f64)
    tkp = np.asarray(tgt_keypoints, dtype=f64)
    rot = np.asarray(rotation_ab, dtype=f64)
    tra = np.asarray(translation_ab, dtype=f64)
    sknn = np.asarray(src_keypoints_knn, dtype=f64)
    tknn = np.asarray(tgt_keypoints_knn, dtype=f64)

    transformed = np.einsum('bij,bjk->bik', rot, skp) + tra[:, :, None]
    kp_sq = (transformed - tkp) ** 2
    keypoints_loss = np.mean(np.sum(kp_sq, axis=(1, 2)))
    knn_sq = (sknn - tknn) ** 2
    knn_loss = np.mean(np.sum(knn_sq, axis=(1, 2)))
    return knn_loss + keypoints_loss


def kernel(src_keypoints, tgt_keypoints, rotation_ab, translation_ab,
           src_keypoints_knn, tgt_keypoints_knn, k, src_transformed, tgt,
           **_unused):
    in_maps = _prepare_in_maps(src_transformed, tgt)
    res = run_device(in_maps)
    gal = _postprocess(res.results)
    ncl = _ncl_host(src_keypoints, tgt_keypoints, rotation_ab, translation_ab,
                    src_keypoints_knn, tgt_keypoints_knn)
    return np.float32(ncl), np.float32(gal)


# revision 12
# speedup vs baseline: 2.0459x; 1.0246x over previous
"""RIENet loss kernel (keypoint/KNN MSE + global-align Huber-min loss) on 8 trn2 cores.

Sharding: core ci -> (b = ci // 4, n-chunk j = ci % 4).  Each core holds the full
tgt[b] (M=8192 points) and a 2048-column chunk of src_transformed[b] (N axis).
  loss_1 (min over M per src point): complete locally per core.
  loss_2 (min over N per tgt point): per-core partial min over its chunk;
          host min-reduces the 4 chunks per batch element.

v3 design (three-engine pipeline, host-side operand prep):
  Host builds bf16-split operand matrices tA [24, M] / sA [24, CHUNK] so that a
  single K=24 bf16 matmul produces the FULL squared-distance matrix
    P[m, n] = ||t_m||^2 + ||s_n||^2 - 2 t_m.s_n
  in fp32 PSUM (3-way bf16 splits of t, s and both norms; the 6 dominant
  cross products; abs err ~1e-5).  No device-side splits/transposes.

  Per 128-row m-tile (64 tiles):
    PE:   4 x 512-col bf16 matmuls -> PSUM [128, 2048]
    ACT:  copy PSUM -> SBUF bf16 (the only engine that is otherwise idle)
    DVE:  tensor_tensor min          acc_bf = min(acc_bf, q_bf)   (colmin, 2x)
          tensor_mask_reduce min     rowbuf[:, mi] = min_n q_bf   (rowmin)
  Tail: partition-min of acc_bf via 16 PE transposes + DVE reduces.
  Tiny keypoint/KNN MSE losses are computed on host in float64.
"""

import os
import numpy as np


def _ensure_path():
    try:
        import concourse  # noqa: F401
    except ImportError:
        import sys
        for p in ("/opt/trn_rl_repo", "/root/.axon_site/_ro/trn_rl_repo"):
            if os.path.isdir(p) and p not in sys.path:
                sys.path.insert(0, p)


_ensure_path()

import concourse.bass as bass  # noqa: E402
import concourse.bacc as bacc  # noqa: E402
import concourse.tile as tile  # noqa: E402
import concourse.mybir as mybir  # noqa: E402
from concourse.bass_utils import run_bass_kernel_spmd  # noqa: E402

F32 = mybir.dt.float32
BF16 = mybir.dt.bfloat16
AL = mybir.AluOpType

MARGIN = 0.1
B, KP, KNN, N, M = 2, 256, 32, 8192, 8192
NCORES = 8
NSHARDS = NCORES // B          # 4 n-chunks per batch element
CHUNK = N // NSHARDS           # 2048
NJ = CHUNK // 512              # 4 psum banks per m-tile
MI = M // 128                  # 64 m-tiles
K24 = 24
BIG = 3.0e38

_CACHE = {}
_BF16_NP = mybir.dt.np(BF16)


def _build():
    nc = bacc.Bacc("TRN2", target_bir_lowering=False, debug=False,
                   num_devices=NCORES)

    tA_d = nc.dram_tensor("tA", [K24, M], BF16, kind="ExternalInput")
    sA_d = nc.dram_tensor("sA", [K24, CHUNK], BF16, kind="ExternalInput")
    ident = nc.dram_tensor("ident", [128, 128], F32, kind="ExternalInput")

    colmin_o = nc.dram_tensor("colmin", [128, CHUNK // 128], F32,
                              kind="ExternalOutput")
    rowmin_o = nc.dram_tensor("rowmin", [128, MI], F32, kind="ExternalOutput")

    with tile.TileContext(nc) as tc:
        with (
            tc.tile_pool(name="const", bufs=1) as const,
            tc.tile_pool(name="qp", bufs=2) as qp,
        ):
            tA = const.tile([K24, M], BF16)
            sA = const.tile([K24, CHUNK], BF16)
            id_sb = const.tile([128, 128], F32)
            accf32 = const.tile([128, CHUNK], F32)
            accbf = const.tile([128, CHUNK], BF16)
            rowq = const.tile([128, MI, 1024], BF16)
            rowbuf = const.tile([128, MI], F32)
            colmin_sb = const.tile([128, CHUNK // 128], F32)

            nc.sync.dma_start(out=tA[:], in_=tA_d[:])
            nc.sync.dma_start(out=sA[:], in_=sA_d[:])
            nc.sync.dma_start(out=id_sb[:], in_=ident[:])
            nc.gpsimd.memset(accbf[:], BIG)

            # ---- main loop: P = nt + ns - 2 t.s per 128-row m-tile ----
            with tc.tile_pool(name="psum_main", bufs=2, space="PSUM") as pm:
                for mi in range(MI):
                    pt = pm.tile([128, CHUNK], F32, tag="pt")
                    for nj in range(NJ):
                        nc.tensor.matmul(
                            pt[:, nj * 512:(nj + 1) * 512],
                            lhsT=tA[:, mi * 128:(mi + 1) * 128],
                            rhs=sA[:, nj * 512:(nj + 1) * 512],
                            start=True, stop=True,
                        )
                    qbf = qp.tile([128, CHUNK], BF16, tag="qbf")
                    nc.scalar.copy(out=qbf[:], in_=pt[:])
                    # rowmin fold 2048 -> 1024 into this tile's rowq slot
                    nc.vector.tensor_tensor(
                        out=rowq[:, mi, :], in0=qbf[:, 0:1024],
                        in1=qbf[:, 1024:2048], op=AL.min)
                    # colmin partial: acc = min(acc, q)
                    nc.vector.tensor_tensor(
                        out=accbf[:], in0=qbf[:], in1=accbf[:], op=AL.min)
                    # interleaved batch rowmin: after each group of 8 tiles,
                    # halve those 8 rowq slots 1024 -> 16 in place
                    if mi % 8 == 7:
                        g = mi - 7
                        w = 1024
                        while w > 16:
                            h = w // 2
                            nc.vector.tensor_tensor(
                                out=rowq[:, g:g + 8, 0:h],
                                in0=rowq[:, g:g + 8, 0:h],
                                in1=rowq[:, g:g + 8, h:w], op=AL.min)
                            w = h

                nc.vector.tensor_reduce(
                    out=rowbuf[:], in_=rowq[:, :, 0:16],
                    axis=mybir.AxisListType.X, op=AL.min)
            nc.sync.dma_start(out=rowmin_o[:], in_=rowbuf[:])

            with tc.tile_pool(name="psum_fin", bufs=1, space="PSUM") as pf:
                # partition-axis min of accbf via PE transposes (fp32 path)
                nc.scalar.copy(out=accf32[:], in_=accbf[:])
                tp = pf.tile([128, CHUNK // 128, 128], F32)
                for blk in range(CHUNK // 128):
                    nc.tensor.transpose(tp[:, blk, :],
                                        accf32[:, blk * 128:(blk + 1) * 128],
                                        id_sb[:])
                nc.vector.tensor_reduce(
                    out=colmin_sb[:], in_=tp[:],
                    axis=mybir.AxisListType.X, op=AL.min)

            nc.sync.dma_start(out=colmin_o[:], in_=colmin_sb[:])

    nc.compile()
    return nc


def _get_nc():
    if "nc" not in _CACHE:
        _CACHE["nc"] = _build()
    return _CACHE["nc"]


def _split3(x):
    """3-way bf16 split of a float array (computed in float64/float32)."""
    x = np.asarray(x, dtype=np.float64)
    h = x.astype(_BF16_NP)
    r1 = x - h.astype(np.float64)
    m = r1.astype(_BF16_NP)
    r2 = r1 - m.astype(np.float64)
    l = r2.astype(_BF16_NP)
    return h, m, l


def _prepare_in_maps(src_transformed, tgt):
    f = np.float32
    st = np.asarray(src_transformed, dtype=f)
    tg = np.asarray(tgt, dtype=f)

    ident = np.eye(128, dtype=f)

    in_maps = []
    for ci in range(NCORES):
        b, j = divmod(ci, NSHARDS)
        t = tg[b]                                    # (3, M)
        s = st[b, :, j * CHUNK:(j + 1) * CHUNK]      # (3, CHUNK)

        th, tm, tl = _split3(t)
        sh, sm, sl = _split3(s)
        nt = np.sum(t.astype(np.float64) ** 2, axis=0)
        ns = np.sum(s.astype(np.float64) ** 2, axis=0)
        nth, ntm, ntl = _split3(nt)
        nsh, nsm, nsl = _split3(ns)

        def neg2(a):
            return (a.astype(f) * -2.0).astype(_BF16_NP)

        tA = np.zeros((K24, M), dtype=_BF16_NP)
        sA = np.zeros((K24, CHUNK), dtype=_BF16_NP)
        # cross products: (th,sh) (th,sm) (tm,sh) (tm,sm) (th,sl) (tl,sh)
        pairs = [(th, sh), (th, sm), (tm, sh), (tm, sm), (th, sl), (tl, sh)]
        r = 0
        for tp_, sp_ in pairs:
            for d in range(3):
                tA[r] = neg2(tp_[d])
                sA[r] = sp_[d]
                r += 1
        # ||s||^2 rows: ones x ns splits
        for part in (nsh, nsm, nsl):
            tA[r] = np.ones(M, dtype=_BF16_NP)
            sA[r] = part
            r += 1
        # ||t||^2 rows: nt splits x ones
        for part in (nth, ntm, ntl):
            tA[r] = part
            sA[r] = np.ones(CHUNK, dtype=_BF16_NP)
            r += 1
        assert r == K24

        in_maps.append({
            "tA": np.ascontiguousarray(tA),
            "sA": np.ascontiguousarray(sA),
            "ident": ident,
        })
    return in_maps


def _huber(x, c):
    return np.where(x < c, 0.5 * x * x, c * x - 0.5 * c * c)


def _postprocess(results):
    c = np.float64(MARGIN)
    loss1 = np.float64(0.0)
    loss2 = np.float64(0.0)
    for b in range(B):
        rowmins = []
        for j in range(NSHARDS):
            r = results[b * NSHARDS + j]
            colmin = np.asarray(r["colmin"], dtype=np.float64).T.ravel()
            loss1 += _huber(colmin, c).sum()
            rowmins.append(np.asarray(r["rowmin"], dtype=np.float64).T.ravel())
        rm = np.minimum.reduce(rowmins)
        loss2 += _huber(rm, c).sum()
    return loss1 + loss2


def run_device(in_maps, **kw):
    nc = _get_nc()
    return run_bass_kernel_spmd(nc, in_maps, list(range(NCORES)), **kw)


def _ncl_host(src_keypoints, tgt_keypoints, rotation_ab, translation_ab,
              src_keypoints_knn, tgt_keypoints_knn):
    f64 = np.float64
    skp = np.asarray(src_keypoints, dtype=f64)
    tkp = np.asarray(tgt_keypoints, dtype=f64)
    rot = np.asarray(rotation_ab, dtype=f64)
    tra = np.asarray(translation_ab, dtype=f64)
    sknn = np.asarray(src_keypoints_knn, dtype=f64)
    tknn = np.asarray(tgt_keypoints_knn, dtype=f64)

    transformed = np.einsum('bij,bjk->bik', rot, skp) + tra[:, :, None]
    kp_sq = (transformed - tkp) ** 2
    keypoints_loss = np.mean(np.sum(kp_sq, axis=(1, 2)))
    knn_sq = (sknn - tknn) ** 2
    knn_loss = np.mean(np.sum(knn_sq, axis=(1, 2)))
    return knn_loss + keypoints_loss


def kernel(src_keypoints, tgt_keypoints, rotation_ab, translation_ab,
           src_keypoints_knn, tgt_keypoints_knn, k, src_transformed, tgt,
           **_unused):
    in_maps = _prepare_in_maps(src_transformed, tgt)
    res = run_device(in_maps)
    gal = _postprocess(res.results)
    ncl = _ncl_host(src_keypoints, tgt_keypoints, rotation_ab, translation_ab,
                    src_keypoints_knn, tgt_keypoints_knn)
    return np.float32(ncl), np.float32(gal)


# revision 22
# speedup vs baseline: 2.1205x; 1.0365x over previous
"""RIENet loss kernel (keypoint/KNN MSE + global-align Huber-min loss) on 8 trn2 cores.

Sharding: core ci -> (b = ci // 4, n-chunk j = ci % 4).  Each core holds the full
tgt[b] (M=8192 points) and a 2048-column chunk of src_transformed[b] (N axis).
  loss_1 (min over M per src point): complete locally per core.
  loss_2 (min over N per tgt point): per-core partial min over its chunk;
          host min-reduces the 4 chunks per batch element.

v3 design (three-engine pipeline, host-side operand prep):
  Host builds bf16-split operand matrices tA [24, M] / sA [24, CHUNK] so that a
  single K=24 bf16 matmul produces the FULL squared-distance matrix
    P[m, n] = ||t_m||^2 + ||s_n||^2 - 2 t_m.s_n
  in fp32 PSUM (3-way bf16 splits of t, s and both norms; the 6 dominant
  cross products; abs err ~1e-5).  No device-side splits/transposes.

  Per 128-row m-tile (64 tiles):
    PE:   4 x 512-col bf16 matmuls -> PSUM [128, 2048]
    ACT:  copy PSUM -> SBUF bf16 (the only engine that is otherwise idle)
    DVE:  tensor_tensor min          acc_bf = min(acc_bf, q_bf)   (colmin, 2x)
          tensor_mask_reduce min     rowbuf[:, mi] = min_n q_bf   (rowmin)
  Tail: partition-min of acc_bf via 16 PE transposes + DVE reduces.
  Tiny keypoint/KNN MSE losses are computed on host in float64.
"""

import os
import numpy as np


def _ensure_path():
    try:
        import concourse  # noqa: F401
    except ImportError:
        import sys
        for p in ("/opt/trn_rl_repo", "/root/.axon_site/_ro/trn_rl_repo"):
            if os.path.isdir(p) and p not in sys.path:
                sys.path.insert(0, p)


_ensure_path()

import concourse.bass as bass  # noqa: E402
import concourse.bacc as bacc  # noqa: E402
import concourse.tile as tile  # noqa: E402
import concourse.mybir as mybir  # noqa: E402
from concourse.bass_utils import run_bass_kernel_spmd  # noqa: E402

F32 = mybir.dt.float32
BF16 = mybir.dt.bfloat16
AL = mybir.AluOpType

MARGIN = 0.1
B, KP, KNN, N, M = 2, 256, 32, 8192, 8192
NCORES = 8
NSHARDS = NCORES // B          # 4 n-chunks per batch element
CHUNK = N // NSHARDS           # 2048
NJ = CHUNK // 512              # 4 psum banks per m-tile
MI = M // 128                  # 64 m-tiles
K24 = 24
BIG = 3.0e38

_CACHE = {}
_BF16_NP = mybir.dt.np(BF16)


def _build():
    nc = bacc.Bacc("TRN2", target_bir_lowering=False, debug=False,
                   num_devices=NCORES)

    tA_d = nc.dram_tensor("tA", [K24, M], BF16, kind="ExternalInput")
    sA_d = nc.dram_tensor("sA", [K24, CHUNK], BF16, kind="ExternalInput")
    ident = nc.dram_tensor("ident", [128, 128], F32, kind="ExternalInput")

    colmin_o = nc.dram_tensor("colmin", [128, CHUNK // 128], F32,
                              kind="ExternalOutput")
    rowmin_o = nc.dram_tensor("rowmin", [128, MI], F32, kind="ExternalOutput")

    with tile.TileContext(nc) as tc:
        with (
            tc.tile_pool(name="const", bufs=1) as const,
            tc.tile_pool(name="qp", bufs=3) as qp,
        ):
            tA = const.tile([K24, M], BF16)
            sA = const.tile([K24, CHUNK], BF16)
            id_sb = const.tile([128, 128], F32)
            accf32 = const.tile([128, CHUNK], F32)
            accbf = const.tile([128, CHUNK], BF16)
            rowq = const.tile([128, MI, 1024], BF16)
            rowbuf = const.tile([128, MI], F32)
            colmin_sb = const.tile([128, CHUNK // 128], F32)

            nc.sync.dma_start(out=sA[:], in_=sA_d[:])
            nc.sync.dma_start(out=tA[:, 0:1024], in_=tA_d[:, 0:1024])
            nc.sync.dma_start(out=tA[:, 1024:M], in_=tA_d[:, 1024:M])
            nc.sync.dma_start(out=id_sb[:], in_=ident[:])
            nc.gpsimd.memset(accbf[:], BIG)

            # ---- main loop: P = nt + ns - 2 t.s per 128-row m-tile ----
            with tc.tile_pool(name="psum_main", bufs=2, space="PSUM") as pm:
                for mi in range(MI):
                    pt = pm.tile([128, CHUNK], F32, tag="pt")
                    for nj in range(NJ):
                        nc.tensor.matmul(
                            pt[:, nj * 512:(nj + 1) * 512],
                            lhsT=tA[:, mi * 128:(mi + 1) * 128],
                            rhs=sA[:, nj * 512:(nj + 1) * 512],
                            start=True, stop=True,
                        )
                    qbf = qp.tile([128, CHUNK], BF16, tag="qbf")
                    nc.scalar.copy(out=qbf[:], in_=pt[:])
                    # rowmin fold 2048 -> 1024 into this tile's rowq slot
                    nc.vector.tensor_tensor(
                        out=rowq[:, mi, :], in0=qbf[:, 0:1024],
                        in1=qbf[:, 1024:2048], op=AL.min)
                    # colmin partial: acc = min(acc, q)
                    nc.vector.tensor_tensor(
                        out=accbf[:], in0=qbf[:], in1=accbf[:], op=AL.min)
                    # interleaved batch rowmin: after each group of 16 tiles,
                    # halve those 16 rowq slots 1024 -> 16 in place
                    if mi % 16 == 15:
                        g = mi - 15
                        w = 1024
                        while w > 16:
                            h = w // 2
                            nc.vector.tensor_tensor(
                                out=rowq[:, g:g + 16, 0:h],
                                in0=rowq[:, g:g + 16, 0:h],
                                in1=rowq[:, g:g + 16, h:w], op=AL.min)
                            w = h

                nc.vector.tensor_reduce(
                    out=rowbuf[:], in_=rowq[:, :, 0:16],
                    axis=mybir.AxisListType.X, op=AL.min)
            nc.sync.dma_start(out=rowmin_o[:], in_=rowbuf[:])

            with tc.tile_pool(name="psum_fin", bufs=1, space="PSUM") as pf:
                # partition-axis min of accbf via PE transposes (fp32 path)
                nc.scalar.copy(out=accf32[:], in_=accbf[:])
                tp = pf.tile([128, CHUNK // 128, 128], F32)
                for blk in range(CHUNK // 128):
                    nc.tensor.transpose(tp[:, blk, :],
                                        accf32[:, blk * 128:(blk + 1) * 128],
                                        id_sb[:])
                nc.vector.tensor_reduce(
                    out=colmin_sb[:], in_=tp[:],
                    axis=mybir.AxisListType.X, op=AL.min)

            nc.sync.dma_start(out=colmin_o[:], in_=colmin_sb[:])

    nc.compile()
    return nc


def _get_nc():
    if "nc" not in _CACHE:
        _CACHE["nc"] = _build()
    return _CACHE["nc"]


def _split3(x):
    """3-way bf16 split of a float array (computed in float64/float32)."""
    x = np.asarray(x, dtype=np.float64)
    h = x.astype(_BF16_NP)
    r1 = x - h.astype(np.float64)
    m = r1.astype(_BF16_NP)
    r2 = r1 - m.astype(np.float64)
    l = r2.astype(_BF16_NP)
    return h, m, l


def _prepare_in_maps(src_transformed, tgt):
    f = np.float32
    st = np.asarray(src_transformed, dtype=f)
    tg = np.asarray(tgt, dtype=f)

    ident = np.eye(128, dtype=f)

    in_maps = []
    for ci in range(NCORES):
        b, j = divmod(ci, NSHARDS)
        t = tg[b]                                    # (3, M)
        s = st[b, :, j * CHUNK:(j + 1) * CHUNK]      # (3, CHUNK)

        th, tm, tl = _split3(t)
        sh, sm, sl = _split3(s)
        nt = np.sum(t.astype(np.float64) ** 2, axis=0)
        ns = np.sum(s.astype(np.float64) ** 2, axis=0)
        nth, ntm, ntl = _split3(nt)
        nsh, nsm, nsl = _split3(ns)

        def neg2(a):
            return (a.astype(f) * -2.0).astype(_BF16_NP)

        tA = np.zeros((K24, M), dtype=_BF16_NP)
        sA = np.zeros((K24, CHUNK), dtype=_BF16_NP)
        # cross products: (th,sh) (th,sm) (tm,sh) (tm,sm) (th,sl) (tl,sh)
        pairs = [(th, sh), (th, sm), (tm, sh), (tm, sm), (th, sl), (tl, sh)]
        r = 0
        for tp_, sp_ in pairs:
            for d in range(3):
                tA[r] = neg2(tp_[d])
                sA[r] = sp_[d]
                r += 1
        # ||s||^2 rows: ones x ns splits
        for part in (nsh, nsm, nsl):
            tA[r] = np.ones(M, dtype=_BF16_NP)
            sA[r] = part
            r += 1
        # ||t||^2 rows: nt splits x ones
        for part in (nth, ntm, ntl):
            tA[r] = part
            sA[r] = np.ones(CHUNK, dtype=_BF16_NP)
            r += 1
        assert r == K24

        in_maps.append({
            "tA": np.ascontiguousarray(tA),
            "sA": np.ascontiguousarray(sA),
            "ident": ident,
        })
    return in_maps


def _huber(x, c):
    return np.where(x < c, 0.5 * x * x, c * x - 0.5 * c * c)


def _postprocess(results):
    c = np.float64(MARGIN)
    loss1 = np.float64(0.0)
    loss2 = np.float64(0.0)
    for b in range(B):
        rowmins = []
        for j in range(NSHARDS):
            r = results[b * NSHARDS + j]
            colmin = np.asarray(r["colmin"], dtype=np.float64).T.ravel()
            loss1 += _huber(colmin, c).sum()
            rowmins.append(np.asarray(r["rowmin"], dtype=np.float64).T.ravel())
        rm = np.minimum.reduce(rowmins)
        loss2 += _huber(rm, c).sum()
    return loss1 + loss2


def run_device(in_maps, **kw):
    nc = _get_nc()
    return run_bass_kernel_spmd(nc, in_maps, list(range(NCORES)), **kw)


def _ncl_host(src_keypoints, tgt_keypoints, rotation_ab, translation_ab,
              src_keypoints_knn, tgt_keypoints_knn):
    f64 = np.float64
    skp = np.asarray(src_keypoints, dtype=f64)
    tkp = np.asarray(tgt_keypoints, dtype=f64)
    rot = np.asarray(rotation_ab, dtype=f64)
    tra = np.asarray(translation_ab, dtype=f64)
    sknn = np.asarray(src_keypoints_knn, dtype=f64)
    tknn = np.asarray(tgt_keypoints_knn, dtype=f64)

    transformed = np.einsum('bij,bjk->bik', rot, skp) + tra[:, :, None]
    kp_sq = (transformed - tkp) ** 2
    keypoints_loss = np.mean(np.sum(kp_sq, axis=(1, 2)))
    knn_sq = (sknn - tknn) ** 2
    knn_loss = np.mean(np.sum(knn_sq, axis=(1, 2)))
    return knn_loss + keypoints_loss


def kernel(src_keypoints, tgt_keypoints, rotation_ab, translation_ab,
           src_keypoints_knn, tgt_keypoints_knn, k, src_transformed, tgt,
           **_unused):
    in_maps = _prepare_in_maps(src_transformed, tgt)
    res = run_device(in_maps)
    gal = _postprocess(res.results)
    ncl = _ncl_host(src_keypoints, tgt_keypoints, rotation_ab, translation_ab,
                    src_keypoints_knn, tgt_keypoints_knn)
    return np.float32(ncl), np.float32(gal)
